# revision 3
# baseline (speedup 1.0000x reference)
"""Coattention layer (nn_CoattentiveLayer) as a Trainium2 Bass/Tile kernel.

Full-input contract: kernel(p, q, p_masks, q_masks, W) with
  p [16,1024,768] f32, q [16,512,768] f32, p_masks [16,1024] bool,
  q_masks [16,512] bool, W [768,768] f32
returns (p_to_q_attn_vec [16,1024,1536] f32, q_to_p_attn_vec [16,512,768] f32),
matching reference jax semantics. Data-parallel over batch: 16 batches are
sharded 2-per-core across 8 NeuronCores; no cross-core communication.

Per-core program (per batch):
  q_projT = W^T-chunks @ q^T           (PE; W^T built once via PE transposes)
  A^T and A by matmul from (pT, q_projT), with the -1e8 padding-mask biases
  folded in as a rank-1 (K=1) accumulation into the same PSUM group.
  softmax over the free axis in each layout: reduce_max(negate) + one ACT
  Exp pass with accum_out row-sum. Normalization deferred: 1/sum becomes a
  per-partition scale fused into the PSUM->SBUF copy of the output matmuls.
  attn matrices transposed back with PE transposes, then
  q_to_p = attn_p^T @ p, p_to_q = attn_q^T @ [q | q_to_p] (concat = split rhs).

Matmuls run as float32r (full-rate fp32 PE path, ~11-bit mantissa). The BIR
verifier requires every producer of an f32r matmul operand to round to f32r,
so those tiles (and the p/q/W DRAM views - same bits as f32) are declared
float32r and the PSUM->SBUF copies do the rounding.
"""
import sys

import numpy as np

try:
    import concourse.bass as bass  # noqa: F401
except Exception:  # pragma: no cover - path fallback
    for _p in ("/root/.axon_site/_ro/trn_rl_repo", "/opt/trn_rl_repo"):
        if _p not in sys.path:
            sys.path.append(_p)

import concourse.tile as tile
from concourse import bacc, mybir
from concourse.bass_utils import run_bass_kernel_spmd
from concourse.masks import make_identity

F32 = mybir.dt.float32
F32R = mybir.dt.float32r
U8 = mybir.dt.uint8

N_CORES = 8
B = 16
B_PER_CORE = B // N_CORES
P, Q, D = 1024, 512, 768
PC, QC, DC = P // 128, Q // 128, D // 128  # 8, 4, 6
C = D + D  # 1536 concat dim
HUGE = 1e8


def build_nc():
    nc = bacc.Bacc("TRN2", target_bir_lowering=False)

    p_dram = nc.dram_tensor("p", [B_PER_CORE, P, D], F32R, kind="ExternalInput")
    q_dram = nc.dram_tensor("q", [B_PER_CORE, Q, D], F32R, kind="ExternalInput")
    pm_dram = nc.dram_tensor("pm", [B_PER_CORE, P], U8, kind="ExternalInput")
    qm_dram = nc.dram_tensor("qm", [B_PER_CORE, Q], U8, kind="ExternalInput")
    w_dram = nc.dram_tensor("w", [D, D], F32R, kind="ExternalInput")
    o1_dram = nc.dram_tensor("o1", [B_PER_CORE, P, C], F32, kind="ExternalOutput")
    o2_dram = nc.dram_tensor("o2", [B_PER_CORE, Q, D], F32, kind="ExternalOutput")

    with tile.TileContext(nc) as tc:
        with (
            tc.tile_pool(name="consts", bufs=1) as consts,
            tc.tile_pool(name="big", bufs=2) as big,       # W_nat/p_nat rotate
            tc.tile_pool(name="mats", bufs=1) as mats,     # per-batch matrices
            tc.tile_pool(name="soft", bufs=2) as soft,     # exp matrices M1/M2
            tc.tile_pool(name="small", bufs=4) as small,   # stats rows etc
            tc.tile_pool(name="out", bufs=2) as outp,      # output staging
            tc.tile_pool(name="ps1", bufs=4, space="PSUM") as ps1,  # 1-bank tiles
            tc.tile_pool(name="ps2", bufs=2, space="PSUM") as ps2,  # 2-bank tiles
        ):
            ident_f = consts.tile([128, 128], F32)
            make_identity(nc, ident_f)
            ident_r = consts.tile([128, 128], F32R)
            nc.vector.tensor_copy(ident_r, ident_f)
            ones_f = consts.tile([1, 128], F32)
            nc.vector.memset(ones_f, 1.0)
            ones_row = consts.tile([1, 128], F32R)
            nc.vector.tensor_copy(ones_row, ones_f)

            # ---- W^T, built once: wt[:, k, :] = W.T[k*128:(k+1)*128, :] ----
            w_nat = big.tile([128, PC, D], F32R, tag="big")  # only [:, :6, :] used
            nc.sync.dma_start(
                out=w_nat[:, :DC, :],
                in_=w_dram.rearrange("(c p) d -> p c d", p=128),
            )
            wt = consts.tile([128, DC, D], F32R)
            for k in range(DC):  # qd chunk (partition of wt)
                for m in range(DC):  # pd chunk (free of wt)
                    pt = ps1.tile([128, 128], F32R, tag="ps1")
                    nc.tensor.transpose(
                        pt, w_nat[:, m, k * 128:(k + 1) * 128], ident_r
                    )
                    nc.any.tensor_copy(wt[:, k, m * 128:(m + 1) * 128], pt)

            for b in range(B_PER_CORE):
                # ---- load inputs ----
                p_nat = big.tile([128, PC, D], F32R, tag="big")
                nc.sync.dma_start(
                    out=p_nat, in_=p_dram[b].rearrange("(c p) d -> p c d", p=128)
                )
                q_nat = mats.tile([128, QC, D], F32R, tag="qnat")
                nc.sync.dma_start(
                    out=q_nat, in_=q_dram[b].rearrange("(c p) d -> p c d", p=128)
                )

                # mask bias rows: -1e8 where masked
                pm_u8 = small.tile([1, P], U8, tag="pmu8", bufs=2)
                nc.sync.dma_start(out=pm_u8, in_=pm_dram[b][None, :])
                pbias = small.tile([1, P], F32R, tag="pbias", bufs=2)
                nc.vector.tensor_copy(pbias, pm_u8)
                nc.vector.tensor_scalar_mul(pbias, pbias, -HUGE)
                qm_u8 = small.tile([1, Q], U8, tag="qmu8", bufs=2)
                nc.sync.dma_start(out=qm_u8, in_=qm_dram[b][None, :])
                qbias = small.tile([1, Q], F32R, tag="qbias", bufs=2)
                nc.vector.tensor_copy(qbias, qm_u8)
                nc.vector.tensor_scalar_mul(qbias, qbias, -HUGE)

                # ---- transposes pT, qT ----
                p_t = mats.tile([128, DC, P], F32R, tag="pt")
                for c in range(PC):
                    for j in range(DC):
                        pt = ps1.tile([128, 128], F32R, tag="ps1")
                        nc.tensor.transpose(
                            pt, p_nat[:, c, j * 128:(j + 1) * 128], ident_r
                        )
                        nc.any.tensor_copy(p_t[:, j, c * 128:(c + 1) * 128], pt)
                q_t = mats.tile([128, DC, Q], F32R, tag="qt")
                for c in range(QC):
                    for j in range(DC):
                        pt = ps1.tile([128, 128], F32R, tag="ps1")
                        nc.tensor.transpose(
                            pt, q_nat[:, c, j * 128:(j + 1) * 128], ident_r
                        )
                        nc.any.tensor_copy(q_t[:, j, c * 128:(c + 1) * 128], pt)

                # ---- q_projT[:, m, :] = (q @ W.T).T chunk = sum_k wt[k,m].T @ qT[k]
                q_projt = mats.tile([128, DC, Q], F32R, tag="qprojt")
                for m in range(DC):
                    acc = ps1.tile([128, Q], F32, tag="ps1")
                    for k in range(DC):
                        nc.tensor.matmul(
                            acc,
                            wt[:, k, m * 128:(m + 1) * 128],
                            q_t[:, k, :],
                            start=(k == 0),
                            stop=(k == DC - 1),
                        )
                    nc.any.tensor_copy(q_projt[:, m, :], acc)

                # ---- A^T chunks + softmax over p -> raw exp M1, 1/sum r1 ----
                # attn_p stored back-transposed: attn_p[:, pc, qc*128:...]
                attn_p = mats.tile([128, PC, Q], F32R, tag="attnp")
                r1_all = small.tile([128, QC], F32, tag="r1")
                for qc in range(QC):
                    at = ps2.tile([128, P], F32, tag="ps2")
                    for h in range(2):
                        hs = slice(h * 512, (h + 1) * 512)
                        for k in range(DC):
                            nc.tensor.matmul(
                                at[:, hs],
                                q_projt[:, k, qc * 128:(qc + 1) * 128],
                                p_t[:, k, hs],
                                start=(k == 0),
                                stop=False,
                            )
                        nc.tensor.matmul(
                            at[:, hs],
                            ones_row,
                            pbias[:, hs],
                            start=False,
                            stop=True,
                        )
                    negmax = small.tile([128, 1], F32, tag="negmax")
                    nc.vector.tensor_reduce(
                        negmax, at, axis=mybir.AxisListType.X,
                        op=mybir.AluOpType.max, negate=True,
                    )
                    m1 = soft.tile([128, P], F32, tag="m")
                    s1 = small.tile([128, 1], F32, tag="s1")
                    nc.scalar.activation(
                        m1, at, mybir.ActivationFunctionType.Exp,
                        bias=negmax, scale=1.0, accum_out=s1,
                    )
                    nc.vector.reciprocal(r1_all[:, qc:qc + 1], s1)
                    for pc in range(PC):
                        pt = ps1.tile([128, 128], F32, tag="ps1")
                        nc.tensor.transpose(
                            pt, m1[:, pc * 128:(pc + 1) * 128], ident_f
                        )
                        nc.any.tensor_copy(
                            attn_p[:, pc, qc * 128:(qc + 1) * 128], pt
                        )

                # ---- A chunks + softmax over q -> raw exp M2, 1/sum r2 ----
                attn_q = mats.tile([128, QC, P], F32R, tag="attnq")
                r2_all = small.tile([128, PC], F32, tag="r2")
                for pc in range(PC):
                    a = ps1.tile([128, Q], F32, tag="ps1")
                    for k in range(DC):
                        nc.tensor.matmul(
                            a,
                            p_t[:, k, pc * 128:(pc + 1) * 128],
                            q_projt[:, k, :],
                            start=(k == 0),
                            stop=False,
                        )
                    nc.tensor.matmul(
                        a, ones_row, qbias, start=False, stop=True
                    )
                    negmax = small.tile([128, 1], F32, tag="negmax")
                    nc.vector.tensor_reduce(
                        negmax, a, axis=mybir.AxisListType.X,
                        op=mybir.AluOpType.max, negate=True,
                    )
                    m2_full = soft.tile([128, P], F32, tag="m")
                    m2 = m2_full[:, :Q]
                    s2 = small.tile([128, 1], F32, tag="s2")
                    nc.scalar.activation(
                        m2, a, mybir.ActivationFunctionType.Exp,
                        bias=negmax, scale=1.0, accum_out=s2,
                    )
                    nc.vector.reciprocal(r2_all[:, pc:pc + 1], s2)
                    for qc in range(QC):
                        pt = ps1.tile([128, 128], F32, tag="ps1")
                        nc.tensor.transpose(
                            pt, m2[:, qc * 128:(qc + 1) * 128], ident_f
                        )
                        nc.any.tensor_copy(
                            attn_q[:, qc, pc * 128:(pc + 1) * 128], pt
                        )

                # ---- q_to_p[qc] = (sum_pc rawattn_p^T @ p) * r1 ----
                qtp_sb = mats.tile([128, QC, D], F32R, tag="qt")  # reuse q_t slot
                for qc in range(QC):
                    acc = ps2.tile([128, D], F32, tag="ps2")
                    for h, hs in enumerate((slice(0, 512), slice(512, D))):
                        for pc in range(PC):
                            nc.tensor.matmul(
                                acc[:, hs],
                                attn_p[:, pc, qc * 128:(qc + 1) * 128],
                                p_nat[:, pc, hs],
                                start=(pc == 0),
                                stop=(pc == PC - 1),
                            )
                    nc.vector.tensor_scalar_mul(
                        qtp_sb[:, qc, :], acc, r1_all[:, qc:qc + 1]
                    )
                    nc.sync.dma_start(
                        out=o2_dram[b, qc * 128:(qc + 1) * 128, :].bitcast(F32R),
                        in_=qtp_sb[:, qc, :],
                    )

                # ---- p_to_q[pc] = (sum_qc rawattn_q^T @ [q | q_to_p]) * r2 ----
                for pc in range(PC):
                    ptq = outp.tile([128, C], F32, tag="ptq")
                    for src, hs, os_ in (
                        ("q", slice(0, 512), slice(0, 512)),
                        ("q", slice(512, D), slice(512, D)),
                        ("t", slice(0, 512), slice(D, D + 512)),
                        ("t", slice(512, D), slice(D + 512, C)),
                    ):
                        acc = ps1.tile([128, 512], F32, tag="ps1")
                        w_ = hs.stop - hs.start
                        for qc in range(QC):
                            rhs = (q_nat if src == "q" else qtp_sb)[:, qc, hs]
                            nc.tensor.matmul(
                                acc[:, :w_],
                                attn_q[:, qc, pc * 128:(pc + 1) * 128],
                                rhs,
                                start=(qc == 0),
                                stop=(qc == QC - 1),
                            )
                        nc.vector.tensor_scalar_mul(
                            ptq[:, os_], acc[:, :w_], r2_all[:, pc:pc + 1]
                        )
                    nc.sync.dma_start(
                        out=o1_dram[b, pc * 128:(pc + 1) * 128, :], in_=ptq
                    )

    nc.finalize()
    return nc


_NC_CACHE = {}


def _get_nc():
    if "nc" not in _NC_CACHE:
        _NC_CACHE["nc"] = build_nc()
    return _NC_CACHE["nc"]


def make_in_maps(p, q, p_masks, q_masks, W):
    p = np.ascontiguousarray(p, dtype=np.float32)
    q = np.ascontiguousarray(q, dtype=np.float32)
    pm = np.ascontiguousarray(p_masks).astype(np.uint8)
    qm = np.ascontiguousarray(q_masks).astype(np.uint8)
    W = np.ascontiguousarray(W, dtype=np.float32)
    in_maps = []
    for i in range(N_CORES):
        s = slice(i * B_PER_CORE, (i + 1) * B_PER_CORE)
        in_maps.append(
            {"p": p[s], "q": q[s], "pm": pm[s], "qm": qm[s], "w": W}
        )
    return in_maps


def assemble_outputs(results):
    o1 = np.concatenate([results[i]["o1"] for i in range(N_CORES)], axis=0)
    o2 = np.concatenate([results[i]["o2"] for i in range(N_CORES)], axis=0)
    return o1, o2


def kernel(p, q, p_masks, q_masks, W):
    nc = _get_nc()
    in_maps = make_in_maps(p, q, p_masks, q_masks, W)
    res = run_bass_kernel_spmd(nc, in_maps, core_ids=list(range(N_CORES)))
    return assemble_outputs(res.results)


# revision 4
# speedup vs baseline: 255.0289x; 255.0289x over previous
"""Coattention layer (nn_CoattentiveLayer) as a Trainium2 Bass/Tile kernel.

Full-input contract: kernel(p, q, p_masks, q_masks, W) with
  p [16,1024,768] f32, q [16,512,768] f32, p_masks [16,1024] bool,
  q_masks [16,512] bool, W [768,768] f32
returns (p_to_q_attn_vec [16,1024,1536] f32, q_to_p_attn_vec [16,512,768] f32),
matching reference jax semantics. Data-parallel over batch: 16 batches are
sharded 2-per-core across 8 NeuronCores; no cross-core communication.

Per-core program (per batch):
  q_projT = W^T-chunks @ q^T           (PE; W^T built once via PE transposes)
  A^T and A by matmul from (pT, q_projT), with the -1e8 padding-mask biases
  folded in as a rank-1 (K=1) accumulation into the same PSUM group.
  softmax over the free axis in each layout: reduce_max(negate) + one ACT
  Exp pass with accum_out row-sum. Normalization deferred: 1/sum becomes a
  per-partition scale fused into the PSUM->SBUF copy of the output matmuls.
  attn matrices transposed back with PE transposes, then
  q_to_p = attn_p^T @ p, p_to_q = attn_q^T @ [q | q_to_p] (concat = split rhs).

Matmuls run as float32r (full-rate fp32 PE path, ~11-bit mantissa). The BIR
verifier requires every producer of an f32r matmul operand to round to f32r,
so those tiles (and the p/q/W DRAM views - same bits as f32) are declared
float32r and the PSUM->SBUF copies do the rounding.
"""
import sys

import numpy as np

try:
    import concourse.bass as bass  # noqa: F401
except Exception:  # pragma: no cover - path fallback
    for _p in ("/root/.axon_site/_ro/trn_rl_repo", "/opt/trn_rl_repo"):
        if _p not in sys.path:
            sys.path.append(_p)

import concourse.tile as tile
from concourse import bacc, mybir
from concourse.bass_utils import run_bass_kernel_spmd
from concourse.masks import make_identity

F32 = mybir.dt.float32
F32R = mybir.dt.float32r
U8 = mybir.dt.uint8

N_CORES = 8
B = 16
B_PER_CORE = B // N_CORES
P, Q, D = 1024, 512, 768
PC, QC, DC = P // 128, Q // 128, D // 128  # 8, 4, 6
C = D + D  # 1536 concat dim
HUGE = 1e8


def build_nc(repeat=1):
    nc = bacc.Bacc("TRN2", target_bir_lowering=False)

    p_dram = nc.dram_tensor("p", [B_PER_CORE, P, D], F32R, kind="ExternalInput")
    q_dram = nc.dram_tensor("q", [B_PER_CORE, Q, D], F32R, kind="ExternalInput")
    pm_dram = nc.dram_tensor("pm", [B_PER_CORE, P], U8, kind="ExternalInput")
    qm_dram = nc.dram_tensor("qm", [B_PER_CORE, Q], U8, kind="ExternalInput")
    w_dram = nc.dram_tensor("w", [D, D], F32R, kind="ExternalInput")
    o1_dram = nc.dram_tensor("o1", [B_PER_CORE, P, C], F32, kind="ExternalOutput")
    o2_dram = nc.dram_tensor("o2", [B_PER_CORE, Q, D], F32, kind="ExternalOutput")

    with tile.TileContext(nc) as tc:
        with (
            tc.tile_pool(name="consts", bufs=1) as consts,
            tc.tile_pool(name="big", bufs=2) as big,       # W_nat/p_nat rotate
            tc.tile_pool(name="mats", bufs=1) as mats,     # per-batch matrices
            tc.tile_pool(name="soft", bufs=2) as soft,     # exp matrices M1/M2
            tc.tile_pool(name="small", bufs=4) as small,   # stats rows etc
            tc.tile_pool(name="out", bufs=2) as outp,      # output staging
            tc.tile_pool(name="ps1", bufs=4, space="PSUM") as ps1,  # 1-bank tiles
            tc.tile_pool(name="ps2", bufs=2, space="PSUM") as ps2,  # 2-bank tiles
        ):
            ident_f = consts.tile([128, 128], F32)
            make_identity(nc, ident_f)
            ident_r = consts.tile([128, 128], F32R)
            nc.vector.tensor_copy(ident_r, ident_f)
            ones_f = consts.tile([1, 128], F32)
            nc.vector.memset(ones_f, 1.0)
            ones_row = consts.tile([1, 128], F32R)
            nc.vector.tensor_copy(ones_row, ones_f)

            # ---- W^T, built once: wt[:, k, :] = W.T[k*128:(k+1)*128, :] ----
            w_nat = big.tile([128, PC, D], F32R, tag="big")  # only [:, :6, :] used
            nc.sync.dma_start(
                out=w_nat[:, :DC, :],
                in_=w_dram.rearrange("(c p) d -> p c d", p=128),
            )
            wt = consts.tile([128, DC, D], F32R)
            for k in range(DC):  # qd chunk (partition of wt)
                for m in range(DC):  # pd chunk (free of wt)
                    pt = ps1.tile([128, 128], F32R, tag="ps1")
                    nc.tensor.transpose(
                        pt, w_nat[:, m, k * 128:(k + 1) * 128], ident_r
                    )
                    nc.any.tensor_copy(wt[:, k, m * 128:(m + 1) * 128], pt)

            for b in [bb % B_PER_CORE for bb in range(repeat * B_PER_CORE)]:
                # ---- load inputs ----
                p_nat = big.tile([128, PC, D], F32R, tag="big")
                nc.sync.dma_start(
                    out=p_nat, in_=p_dram[b].rearrange("(c p) d -> p c d", p=128)
                )
                q_nat = mats.tile([128, QC, D], F32R, tag="qnat")
                nc.sync.dma_start(
                    out=q_nat, in_=q_dram[b].rearrange("(c p) d -> p c d", p=128)
                )

                # mask bias rows: -1e8 where masked
                pm_u8 = small.tile([1, P], U8, tag="pmu8", bufs=2)
                nc.sync.dma_start(out=pm_u8, in_=pm_dram[b][None, :])
                pbias = small.tile([1, P], F32R, tag="pbias", bufs=2)
                nc.vector.tensor_copy(pbias, pm_u8)
                nc.vector.tensor_scalar_mul(pbias, pbias, -HUGE)
                qm_u8 = small.tile([1, Q], U8, tag="qmu8", bufs=2)
                nc.sync.dma_start(out=qm_u8, in_=qm_dram[b][None, :])
                qbias = small.tile([1, Q], F32R, tag="qbias", bufs=2)
                nc.vector.tensor_copy(qbias, qm_u8)
                nc.vector.tensor_scalar_mul(qbias, qbias, -HUGE)

                # ---- transposes pT, qT ----
                p_t = mats.tile([128, DC, P], F32R, tag="pt")
                for c in range(PC):
                    for j in range(DC):
                        pt = ps1.tile([128, 128], F32R, tag="ps1")
                        nc.tensor.transpose(
                            pt, p_nat[:, c, j * 128:(j + 1) * 128], ident_r
                        )
                        nc.any.tensor_copy(p_t[:, j, c * 128:(c + 1) * 128], pt)
                q_t = mats.tile([128, DC, Q], F32R, tag="qt")
                for c in range(QC):
                    for j in range(DC):
                        pt = ps1.tile([128, 128], F32R, tag="ps1")
                        nc.tensor.transpose(
                            pt, q_nat[:, c, j * 128:(j + 1) * 128], ident_r
                        )
                        nc.any.tensor_copy(q_t[:, j, c * 128:(c + 1) * 128], pt)

                # ---- q_projT[:, m, :] = (q @ W.T).T chunk = sum_k wt[k,m].T @ qT[k]
                q_projt = mats.tile([128, DC, Q], F32R, tag="qprojt")
                for m in range(DC):
                    acc = ps1.tile([128, Q], F32, tag="ps1")
                    for k in range(DC):
                        nc.tensor.matmul(
                            acc,
                            wt[:, k, m * 128:(m + 1) * 128],
                            q_t[:, k, :],
                            start=(k == 0),
                            stop=(k == DC - 1),
                        )
                    nc.any.tensor_copy(q_projt[:, m, :], acc)

                # ---- A^T chunks + softmax over p -> raw exp M1, 1/sum r1 ----
                # attn_p stored back-transposed: attn_p[:, pc, qc*128:...]
                attn_p = mats.tile([128, PC, Q], F32R, tag="attnp")
                r1_all = small.tile([128, QC], F32, tag="r1")
                for qc in range(QC):
                    at = ps2.tile([128, P], F32, tag="ps2")
                    for h in range(2):
                        hs = slice(h * 512, (h + 1) * 512)
                        for k in range(DC):
                            nc.tensor.matmul(
                                at[:, hs],
                                q_projt[:, k, qc * 128:(qc + 1) * 128],
                                p_t[:, k, hs],
                                start=(k == 0),
                                stop=False,
                            )
                        nc.tensor.matmul(
                            at[:, hs],
                            ones_row,
                            pbias[:, hs],
                            start=False,
                            stop=True,
                        )
                    negmax = small.tile([128, 1], F32, tag="negmax")
                    nc.vector.tensor_reduce(
                        negmax, at, axis=mybir.AxisListType.X,
                        op=mybir.AluOpType.max, negate=True,
                    )
                    m1 = soft.tile([128, P], F32, tag="m")
                    s1 = small.tile([128, 1], F32, tag="s1")
                    nc.scalar.activation(
                        m1, at, mybir.ActivationFunctionType.Exp,
                        bias=negmax, scale=1.0, accum_out=s1,
                    )
                    nc.vector.reciprocal(r1_all[:, qc:qc + 1], s1)
                    for pc in range(PC):
                        pt = ps1.tile([128, 128], F32, tag="ps1")
                        nc.tensor.transpose(
                            pt, m1[:, pc * 128:(pc + 1) * 128], ident_f
                        )
                        nc.any.tensor_copy(
                            attn_p[:, pc, qc * 128:(qc + 1) * 128], pt
                        )

                # ---- A chunks + softmax over q -> raw exp M2, 1/sum r2 ----
                attn_q = mats.tile([128, QC, P], F32R, tag="attnq")
                r2_all = small.tile([128, PC], F32, tag="r2")
                for pc in range(PC):
                    a = ps1.tile([128, Q], F32, tag="ps1")
                    for k in range(DC):
                        nc.tensor.matmul(
                            a,
                            p_t[:, k, pc * 128:(pc + 1) * 128],
                            q_projt[:, k, :],
                            start=(k == 0),
                            stop=False,
                        )
                    nc.tensor.matmul(
                        a, ones_row, qbias, start=False, stop=True
                    )
                    negmax = small.tile([128, 1], F32, tag="negmax")
                    nc.vector.tensor_reduce(
                        negmax, a, axis=mybir.AxisListType.X,
                        op=mybir.AluOpType.max, negate=True,
                    )
                    m2_full = soft.tile([128, P], F32, tag="m")
                    m2 = m2_full[:, :Q]
                    s2 = small.tile([128, 1], F32, tag="s2")
                    nc.scalar.activation(
                        m2, a, mybir.ActivationFunctionType.Exp,
                        bias=negmax, scale=1.0, accum_out=s2,
                    )
                    nc.vector.reciprocal(r2_all[:, pc:pc + 1], s2)
                    for qc in range(QC):
                        pt = ps1.tile([128, 128], F32, tag="ps1")
                        nc.tensor.transpose(
                            pt, m2[:, qc * 128:(qc + 1) * 128], ident_f
                        )
                        nc.any.tensor_copy(
                            attn_q[:, qc, pc * 128:(pc + 1) * 128], pt
                        )

                # ---- q_to_p[qc] = (sum_pc rawattn_p^T @ p) * r1 ----
                qtp_sb = mats.tile([128, QC, D], F32R, tag="qt")  # reuse q_t slot
                for qc in range(QC):
                    acc = ps2.tile([128, D], F32, tag="ps2")
                    for h, hs in enumerate((slice(0, 512), slice(512, D))):
                        for pc in range(PC):
                            nc.tensor.matmul(
                                acc[:, hs],
                                attn_p[:, pc, qc * 128:(qc + 1) * 128],
                                p_nat[:, pc, hs],
                                start=(pc == 0),
                                stop=(pc == PC - 1),
                            )
                    nc.vector.tensor_scalar_mul(
                        qtp_sb[:, qc, :], acc, r1_all[:, qc:qc + 1]
                    )
                    nc.sync.dma_start(
                        out=o2_dram[b, qc * 128:(qc + 1) * 128, :].bitcast(F32R),
                        in_=qtp_sb[:, qc, :],
                    )

                # ---- p_to_q[pc] = (sum_qc rawattn_q^T @ [q | q_to_p]) * r2 ----
                for pc in range(PC):
                    ptq = outp.tile([128, C], F32, tag="ptq")
                    for src, hs, os_ in (
                        ("q", slice(0, 512), slice(0, 512)),
                        ("q", slice(512, D), slice(512, D)),
                        ("t", slice(0, 512), slice(D, D + 512)),
                        ("t", slice(512, D), slice(D + 512, C)),
                    ):
                        acc = ps1.tile([128, 512], F32, tag="ps1")
                        w_ = hs.stop - hs.start
                        for qc in range(QC):
                            rhs = (q_nat if src == "q" else qtp_sb)[:, qc, hs]
                            nc.tensor.matmul(
                                acc[:, :w_],
                                attn_q[:, qc, pc * 128:(pc + 1) * 128],
                                rhs,
                                start=(qc == 0),
                                stop=(qc == QC - 1),
                            )
                        nc.vector.tensor_scalar_mul(
                            ptq[:, os_], acc[:, :w_], r2_all[:, pc:pc + 1]
                        )
                    nc.sync.dma_start(
                        out=o1_dram[b, pc * 128:(pc + 1) * 128, :], in_=ptq
                    )

    nc.finalize()
    return nc


_NC_CACHE = {}


def _get_nc():
    if "nc" not in _NC_CACHE:
        _NC_CACHE["nc"] = build_nc()
    return _NC_CACHE["nc"]


def make_in_maps(p, q, p_masks, q_masks, W):
    p = np.ascontiguousarray(p, dtype=np.float32)
    q = np.ascontiguousarray(q, dtype=np.float32)
    pm = np.ascontiguousarray(p_masks).astype(np.uint8)
    qm = np.ascontiguousarray(q_masks).astype(np.uint8)
    W = np.ascontiguousarray(W, dtype=np.float32)
    in_maps = []
    for i in range(N_CORES):
        s = slice(i * B_PER_CORE, (i + 1) * B_PER_CORE)
        in_maps.append(
            {"p": p[s], "q": q[s], "pm": pm[s], "qm": qm[s], "w": W}
        )
    return in_maps


def assemble_outputs(results):
    o1 = np.concatenate([results[i]["o1"] for i in range(N_CORES)], axis=0)
    o2 = np.concatenate([results[i]["o2"] for i in range(N_CORES)], axis=0)
    return o1, o2


def kernel(p, q, p_masks, q_masks, W):
    nc = _get_nc()
    in_maps = make_in_maps(p, q, p_masks, q_masks, W)
    res = run_bass_kernel_spmd(nc, in_maps, core_ids=list(range(N_CORES)))
    return assemble_outputs(res.results)


# revision 6
# speedup vs baseline: 1663.4482x; 6.5226x over previous
"""Coattention layer (nn_CoattentiveLayer) as a Trainium2 Bass/Tile kernel.

Full-input contract: kernel(p, q, p_masks, q_masks, W) with
  p [16,1024,768] f32, q [16,512,768] f32, p_masks [16,1024] bool,
  q_masks [16,512] bool, W [768,768] f32
returns (p_to_q_attn_vec [16,1024,1536] f32, q_to_p_attn_vec [16,512,768] f32),
matching reference jax semantics. Data-parallel over batch: 16 batches are
sharded 2-per-core across 8 NeuronCores; no cross-core communication.

Per-core program (per batch):
  q_projT = W^T-chunks @ q^T           (PE; W^T built once via PE transposes)
  A^T and A by matmul from (pT, q_projT), with the -1e8 padding-mask biases
  folded in as a rank-1 (K=1) accumulation into the same PSUM group.
  softmax over the free axis in each layout: reduce_max(negate) + one ACT
  Exp pass with accum_out row-sum. Normalization deferred: 1/sum becomes a
  per-partition scale fused into the PSUM->SBUF copy of the output matmuls.
  attn matrices transposed back with PE transposes (4 blocks batched per
  PSUM bank, one wide strided copy out), then
  q_to_p = attn_p^T @ p, p_to_q = attn_q^T @ [q | q_to_p] (concat = split rhs).

Matmuls run as float32r (full-rate fp32 PE path, ~11-bit mantissa). The BIR
verifier requires every producer of an f32r matmul operand to round to f32r,
so those tiles (and the p/q/W DRAM views - same bits as f32) are declared
float32r and the PSUM->SBUF copies / ACT exp do the rounding.
"""
import sys

import numpy as np

try:
    import concourse.bass as bass  # noqa: F401
except Exception:  # pragma: no cover - path fallback
    for _p in ("/root/.axon_site/_ro/trn_rl_repo", "/opt/trn_rl_repo"):
        if _p not in sys.path:
            sys.path.append(_p)

import concourse.tile as tile
from concourse import bacc, mybir
from concourse.bass_utils import run_bass_kernel_spmd
from concourse.masks import make_identity

F32 = mybir.dt.float32
F32R = mybir.dt.float32r
U8 = mybir.dt.uint8

N_CORES = 8
B = 16
B_PER_CORE = B // N_CORES
P, Q, D = 1024, 512, 768
PC, QC, DC = P // 128, Q // 128, D // 128  # 8, 4, 6
C = D + D  # 1536 concat dim
HUGE = 1e8


def build_nc(repeat=1, exp_f32r=True):
    nc = bacc.Bacc("TRN2", target_bir_lowering=False)

    p_dram = nc.dram_tensor("p", [B_PER_CORE, P, D], F32R, kind="ExternalInput")
    q_dram = nc.dram_tensor("q", [B_PER_CORE, Q, D], F32R, kind="ExternalInput")
    pm_dram = nc.dram_tensor("pm", [B_PER_CORE, P], U8, kind="ExternalInput")
    qm_dram = nc.dram_tensor("qm", [B_PER_CORE, Q], U8, kind="ExternalInput")
    w_dram = nc.dram_tensor("w", [D, D], F32R, kind="ExternalInput")
    o1_dram = nc.dram_tensor("o1", [B_PER_CORE, P, C], F32, kind="ExternalOutput")
    o2_dram = nc.dram_tensor("o2", [B_PER_CORE, Q, D], F32, kind="ExternalOutput")

    MDT = F32R if exp_f32r else F32

    with tile.TileContext(nc) as tc:
        with (
            tc.tile_pool(name="consts", bufs=1) as consts,
            tc.tile_pool(name="big", bufs=2) as big,       # W_nat/p_nat rotate
            tc.tile_pool(name="mats", bufs=1) as mats,     # per-batch matrices
            tc.tile_pool(name="soft", bufs=2) as soft,     # exp matrices M1/M2
            tc.tile_pool(name="small", bufs=4) as small,   # stats rows etc
            tc.tile_pool(name="psT", bufs=2, space="PSUM") as psT,  # transposes
            tc.tile_pool(name="psM", bufs=2, space="PSUM") as psM,  # 1-bank accs
            tc.tile_pool(name="ps2", bufs=2, space="PSUM") as ps2,  # 2-bank accs
        ):
            ident_f = consts.tile([128, 128], F32)
            make_identity(nc, ident_f)
            ident_r = consts.tile([128, 128], F32R)
            nc.vector.tensor_copy(ident_r, ident_f)
            ones_f = consts.tile([1, 128], F32)
            nc.vector.memset(ones_f, 1.0)
            ones_row = consts.tile([1, 128], F32R)
            nc.vector.tensor_copy(ones_row, ones_f)
            ident_m = ident_r if exp_f32r else ident_f

            wt = consts.tile([128, DC, D], F32R)

            for rep in range(repeat):
                for b in range(B_PER_CORE):
                    # ---- load inputs (q first: unblocks PE soonest) ----
                    q_nat = mats.tile([128, QC, D], F32R, tag="qnat", bufs=2)
                    nc.sync.dma_start(
                        out=q_nat,
                        in_=q_dram[b].rearrange("(c p) d -> p c d", p=128),
                    )
                    if rep == 0 and b == 0:
                        w_nat = big.tile([128, PC, D], F32R, tag="big")
                        nc.sync.dma_start(
                            out=w_nat[:, :DC, :],
                            in_=w_dram.rearrange("(c p) d -> p c d", p=128),
                        )
                    p_nat = big.tile([128, PC, D], F32R, tag="big")
                    nc.sync.dma_start(
                        out=p_nat,
                        in_=p_dram[b].rearrange("(c p) d -> p c d", p=128),
                    )

                    # mask bias rows: -1e8 where masked
                    pm_u8 = small.tile([1, P], U8, tag="pmu8", bufs=2)
                    nc.sync.dma_start(out=pm_u8, in_=pm_dram[b][None, :])
                    pbias = small.tile([1, P], F32R, tag="pbias", bufs=2)
                    nc.vector.tensor_copy(pbias, pm_u8)
                    nc.vector.tensor_scalar_mul(pbias, pbias, -HUGE)
                    qm_u8 = small.tile([1, Q], U8, tag="qmu8", bufs=2)
                    nc.sync.dma_start(out=qm_u8, in_=qm_dram[b][None, :])
                    qbias = small.tile([1, Q], F32R, tag="qbias", bufs=2)
                    nc.vector.tensor_copy(qbias, qm_u8)
                    nc.vector.tensor_scalar_mul(qbias, qbias, -HUGE)

                    # ---- qT via PE transposes, 4 blocks per PSUM bank ----
                    q_t = mats.tile([128, DC, Q], F32R, tag="qt")
                    for j in range(DC):
                        pt = psT.tile([128, 512], F32R, tag="psT")
                        for c in range(QC):
                            nc.tensor.transpose(
                                pt[:, c * 128:(c + 1) * 128],
                                q_nat[:, c, j * 128:(j + 1) * 128],
                                ident_r,
                            )
                        nc.any.tensor_copy(q_t[:, j, :], pt)

                    # ---- W^T once: wt[:, k, :] = W.T chunk ----
                    if rep == 0 and b == 0:
                        for k in range(DC):
                            for g in range(2):  # m groups 0-3, 4-5
                                ms = range(4 * g, min(4 * g + 4, DC))
                                wdt = 128 * len(ms)
                                pt = psT.tile([128, 512], F32R, tag="psT")
                                for i, m in enumerate(ms):
                                    nc.tensor.transpose(
                                        pt[:, i * 128:(i + 1) * 128],
                                        w_nat[:, m, k * 128:(k + 1) * 128],
                                        ident_r,
                                    )
                                nc.any.tensor_copy(
                                    wt[:, k, 512 * g:512 * g + wdt], pt[:, :wdt]
                                )

                    # ---- pT ----
                    p_t = mats.tile([128, DC, P], F32R, tag="pt")
                    for j in range(DC):
                        for g in range(2):  # c groups 0-3, 4-7
                            pt = psT.tile([128, 512], F32R, tag="psT")
                            for i in range(4):
                                c = 4 * g + i
                                nc.tensor.transpose(
                                    pt[:, i * 128:(i + 1) * 128],
                                    p_nat[:, c, j * 128:(j + 1) * 128],
                                    ident_r,
                                )
                            nc.any.tensor_copy(
                                p_t[:, j, 512 * g:512 * (g + 1)], pt
                            )

                    # ---- q_projT[:, m, :] = sum_k wt[k,m].T @ qT[k] ----
                    q_projt = mats.tile([128, DC, Q], F32R, tag="qprojt")
                    for m in range(DC):
                        acc = psM.tile([128, Q], F32, tag="psM")
                        for k in range(DC):
                            nc.tensor.matmul(
                                acc,
                                wt[:, k, m * 128:(m + 1) * 128],
                                q_t[:, k, :],
                                start=(k == 0),
                                stop=(k == DC - 1),
                            )
                        nc.any.tensor_copy(q_projt[:, m, :], acc)

                    # ---- A^T chunks + softmax over p -> raw exp M1, 1/sum r1
                    # attn_p stored back-transposed: attn_p[:, pc, qc*128:...]
                    attn_p = mats.tile([128, PC, Q], MDT, tag="attnp")
                    for qc in range(QC):
                        at = ps2.tile([128, P], F32, tag="ps2")
                        for h in range(2):
                            hs = slice(h * 512, (h + 1) * 512)
                            for k in range(DC):
                                nc.tensor.matmul(
                                    at[:, hs],
                                    q_projt[:, k, qc * 128:(qc + 1) * 128],
                                    p_t[:, k, hs],
                                    start=(k == 0),
                                    stop=False,
                                )
                            nc.tensor.matmul(
                                at[:, hs], ones_row, pbias[:, hs],
                                start=False, stop=True,
                            )
                        negmax = small.tile([128, 1], F32, tag="negmax")
                        nc.vector.tensor_reduce(
                            negmax, at, axis=mybir.AxisListType.X,
                            op=mybir.AluOpType.max, negate=True,
                        )
                        m1 = soft.tile([128, P], MDT, tag="m")
                        s1 = small.tile([128, 1], F32, tag="s1")
                        nc.scalar.activation(
                            m1, at, mybir.ActivationFunctionType.Exp,
                            bias=negmax, scale=1.0, accum_out=s1,
                        )
                        r1 = small.tile([128, 1], F32, tag="r1c")
                        nc.vector.reciprocal(r1, s1)
                        nc.vector.tensor_scalar_mul(m1, m1, r1)
                        for g in range(2):  # pc groups 0-3, 4-7
                            pt = psT.tile([128, 512], MDT, tag="psT")
                            for i in range(4):
                                pc = 4 * g + i
                                nc.tensor.transpose(
                                    pt[:, i * 128:(i + 1) * 128],
                                    m1[:, pc * 128:(pc + 1) * 128],
                                    ident_m,
                                )
                            nc.any.tensor_copy(
                                attn_p[:, 4 * g:4 * (g + 1),
                                       qc * 128:(qc + 1) * 128],
                                pt.rearrange("p (a b) -> p a b", a=4),
                            )

                    # ---- A chunks + softmax over q -> raw exp M2, 1/sum r2 --
                    attn_q = mats.tile([128, QC, P], MDT, tag="attnq")
                    for pc in range(PC):
                        a = psM.tile([128, Q], F32, tag="psM")
                        for k in range(DC):
                            nc.tensor.matmul(
                                a,
                                p_t[:, k, pc * 128:(pc + 1) * 128],
                                q_projt[:, k, :],
                                start=(k == 0),
                                stop=False,
                            )
                        nc.tensor.matmul(
                            a, ones_row, qbias, start=False, stop=True
                        )
                        negmax = small.tile([128, 1], F32, tag="negmax")
                        nc.vector.tensor_reduce(
                            negmax, a, axis=mybir.AxisListType.X,
                            op=mybir.AluOpType.max, negate=True,
                        )
                        m2_full = soft.tile([128, P], MDT, tag="m")
                        m2 = m2_full[:, :Q]
                        s2 = small.tile([128, 1], F32, tag="s2")
                        nc.scalar.activation(
                            m2, a, mybir.ActivationFunctionType.Exp,
                            bias=negmax, scale=1.0, accum_out=s2,
                        )
                        r2 = small.tile([128, 1], F32, tag="r2c")
                        nc.vector.reciprocal(r2, s2)
                        nc.vector.tensor_scalar_mul(m2, m2, r2)
                        pt = psT.tile([128, 512], MDT, tag="psT")
                        for qc in range(QC):
                            nc.tensor.transpose(
                                pt[:, qc * 128:(qc + 1) * 128],
                                m2[:, qc * 128:(qc + 1) * 128],
                                ident_m,
                            )
                        nc.any.tensor_copy(
                            attn_q[:, :, pc * 128:(pc + 1) * 128],
                            pt.rearrange("p (a b) -> p a b", a=4),
                        )

                    # ---- q_to_p[qc] = (sum_pc rawattn_p^T @ p) * r1 ----
                    qtp_sb = mats.tile([128, QC, D], F32R, tag="qt")  # q_t slot
                    for qc in range(QC):
                        for h, hs in enumerate((slice(0, 512), slice(512, D))):
                            acc = psM.tile([128, Q], F32, tag="psM")
                            w_ = hs.stop - hs.start
                            for pc in range(PC):
                                nc.tensor.matmul(
                                    acc[:, :w_],
                                    attn_p[:, pc, qc * 128:(qc + 1) * 128],
                                    p_nat[:, pc, hs],
                                    start=(pc == 0),
                                    stop=(pc == PC - 1),
                                )
                            nc.any.tensor_copy(qtp_sb[:, qc, hs], acc[:, :w_])
                        nc.sync.dma_start(
                            out=o2_dram[b, qc * 128:(qc + 1) * 128, :].bitcast(
                                F32R),
                            in_=qtp_sb[:, qc, :],
                        )

                    # ---- p_to_q[pc] = (sum_qc rawattn_q^T @ [q|q_to_p]) * r2
                    for pc in range(PC):
                        for src, hs, os_ in (
                            ("q", slice(0, 512), slice(0, 512)),
                            ("q", slice(512, D), slice(512, D)),
                            ("t", slice(0, 512), slice(D, D + 512)),
                            ("t", slice(512, D), slice(D + 512, C)),
                        ):
                            acc = psM.tile([128, Q], F32, tag="psM")
                            w_ = hs.stop - hs.start
                            for qc in range(QC):
                                rhs = (q_nat if src == "q" else qtp_sb)[:, qc, hs]
                                nc.tensor.matmul(
                                    acc[:, :w_],
                                    attn_q[:, qc, pc * 128:(pc + 1) * 128],
                                    rhs,
                                    start=(qc == 0),
                                    stop=(qc == QC - 1),
                                )
                            nc.sync.dma_start(
                                out=o1_dram[b, pc * 128:(pc + 1) * 128, os_],
                                in_=acc[:, :w_],
                            )

    nc.finalize()
    return nc


_NC_CACHE = {}


def _get_nc():
    if "nc" not in _NC_CACHE:
        _NC_CACHE["nc"] = build_nc()
    return _NC_CACHE["nc"]


def make_in_maps(p, q, p_masks, q_masks, W):
    p = np.ascontiguousarray(p, dtype=np.float32)
    q = np.ascontiguousarray(q, dtype=np.float32)
    pm = np.ascontiguousarray(p_masks).astype(np.uint8)
    qm = np.ascontiguousarray(q_masks).astype(np.uint8)
    W = np.ascontiguousarray(W, dtype=np.float32)
    in_maps = []
    for i in range(N_CORES):
        s = slice(i * B_PER_CORE, (i + 1) * B_PER_CORE)
        in_maps.append(
            {"p": p[s], "q": q[s], "pm": pm[s], "qm": qm[s], "w": W}
        )
    return in_maps


def assemble_outputs(results):
    o1 = np.concatenate([results[i]["o1"] for i in range(N_CORES)], axis=0)
    o2 = np.concatenate([results[i]["o2"] for i in range(N_CORES)], axis=0)
    return o1, o2


def kernel(p, q, p_masks, q_masks, W):
    nc = _get_nc()
    in_maps = make_in_maps(p, q, p_masks, q_masks, W)
    res = run_bass_kernel_spmd(nc, in_maps, core_ids=list(range(N_CORES)))
    return assemble_outputs(res.results)
